# revision 45
# baseline (speedup 1.0000x reference)
"""Multi-head self-attention (RoPE, eval-mode) Trainium2 Bass kernel.

Problem: B=2, T=2048, D=1024, H=16, d_head=64, fp32 I/O.

Sharding (8 cores): core c handles batch b=c//4 and the 4 heads
[4g, 4g+4) where g=c%4.  QKV/attention are head-local; the output
projection produces a per-core partial (contraction over this core's
256 head-dims) which the host sums across the 4 cores of each batch
and adds b_out.

v2 design (vs the two-phase baseline):
  - The ACT exp stream (4 heads * T^2 = 16.8M elems ~ 110us streaming)
    is the hard wall.  The kernel is restructured so exp starts ~16us
    in instead of ~81us: per-quarter rounds emit the K/V/Q projection
    chains interleaved with attention tk-chunks of the first four
    (hp, tq) blocks, which accumulate PV partials into SBUF so the two
    PSUM pv banks don't serialize in-flight blocks.
  - DMA priority: w_qkv chunk 0 + x quarter 0 are issued first and the
    rest in need order, so the first matmul fires at ~6us not ~22us.
  - Scores are issued as two concurrent K=64 row-group matmuls
    (head 0 rows 0:64, head 1 rows 64:128) instead of zero-padded
    K=128 - halves score cycles; RoPE applies in place on the stacked
    k tiles (kstack == stationary source).
  - Emission skews sc one tk ahead of pv so the exp stream stays dense
    across chunk/block boundaries; per-head norm frees pv banks early.
  - PSUM: sc 2x[128,1024] (4 banks) + pv0/pv1 (2) + work ring 2 = 8.
  - v is computed row-major and stored per head as [ones | v] 128-wide
    stationary tiles: each PV matmul yields softmax denominators
    (partitions 0:64) and attn^T (64:128) in one pass.
  - softmax skips max-subtraction (scores ~ N(0,1), exp safe in fp32)
    and normalizes with the fast DVE reciprocal.
"""

import ml_dtypes
import numpy as np

BF16 = ml_dtypes.bfloat16

B, T, D = 2, 2048, 1024
H = 16
DH = 64
NCORES = 8
P = 128

_CACHE = {}
_DBG = False  # debug build: DMA intermediates of block (0,2) to "dbg"
_SKEW = True  # emit sc one tk ahead of the pv flush (denser ACT stream)
_OPTAIL = False  # emit all outproj units at the tail (diagnostic)
_ALLCHUNK = False  # process dense blocks as 4-tk chunks too (diagnostic)


def _rope_tables_np():
    theta = 1.0 / (10000.0 ** (np.arange(0, DH, 2, dtype=np.float32) / DH))
    angles = np.outer(np.arange(T, dtype=np.float32), theta)  # (T, 32)
    angles = np.concatenate([angles, angles], axis=-1)  # (T, DH)
    cos = np.cos(angles).astype(np.float32)
    sin = np.sin(angles).astype(np.float32)
    cosT = np.ascontiguousarray(cos.T)  # (64, T)
    sinT = np.ascontiguousarray(sin.T)
    sinT_signed = np.concatenate([-sinT[0:32], sinT[32:64]], axis=0)
    cos2 = np.tile(cosT, (2, 1))  # (128, T)
    sin2 = np.tile(sinT_signed, (2, 1))
    return cos2, sin2


def _build_module():
    import concourse.mybir as mybir
    import concourse.tile as tile
    from concourse import bacc

    f32 = mybir.dt.float32
    f32r = mybir.dt.float32r
    bf16 = mybir.dt.bfloat16

    nc = bacc.Bacc("TRN2", target_bir_lowering=False, debug=False)
    xT = nc.dram_tensor("xT", [4, P, 8, 512], bf16, kind="ExternalInput")
    w_qk = nc.dram_tensor("w_qk", [P, 8, 512], bf16, kind="ExternalInput")
    w_v = nc.dram_tensor("w_v", [P, 8, 256], bf16, kind="ExternalInput")
    w_o = nc.dram_tensor("w_o", [P, 2, 1024], f32r, kind="ExternalInput")
    cos2 = nc.dram_tensor("cos2", [P, T], bf16, kind="ExternalInput")
    sin2 = nc.dram_tensor("sin2", [P, T], bf16, kind="ExternalInput")
    out = nc.dram_tensor("out", [T, D], bf16, kind="ExternalOutput")
    dbg = (
        nc.dram_tensor("dbg", [20, P, 512], f32, kind="ExternalOutput")
        if _DBG
        else None
    )

    Exp = mybir.ActivationFunctionType.Exp

    # Blocks in completion order.  The first CHUNKED ones accumulate PV
    # into SBUF in 4-tk chunks paced by quarter availability; the rest
    # run dense (16 tk straight, PV resident in PSUM).
    CHUNKED = [(hp, tq) for tq in range(4) for hp in range(2)]

    with tile.TileContext(nc) as tc:
        with tc.tile_pool(name="persist", bufs=1) as persist:
            wqk_sb = persist.tile([P, 8, 512], bf16)
            wv_sb = persist.tile([P, 8, 256], bf16)
            wo_sb = persist.tile([P, 2, 1024], f32r)
            cos_sb = persist.tile([P, T], bf16)
            sin_sb = persist.tile([P, T], bf16)
            # roped q / stacked roped k, two heads per 128-partition tile
            q_q = [
                [persist.tile([P, 512], bf16, tag=f"q{hp}_{t}", name=f"q{hp}_{t}")
                 for t in range(4)]
                for hp in range(2)
            ]
            # zero-padded roped k per head (head h real rows h*64:h*64+64,
            # other half zero) - K=128 score matmuls need no row groups
            kpad = [
                [
                    [persist.tile([P, 512], bf16, tag=f"k{hp}{h}_{t}",
                                  name=f"k{hp}{h}_{t}")
                     for t in range(4)]
                    for h in range(2)
                ]
                for hp in range(2)
            ]
            # per (tk-tile, head): [ones | v] stationary 128x128
            vaug = persist.tile([P, 16, 4, P], bf16)
            attn_q = [
                [persist.tile([P, 512], f32r, tag=f"at{hp}_{b}", name=f"at{hp}_{b}")
                 for b in range(4)]
                for hp in range(2)
            ]
            # SBUF PV accumulators for the chunked blocks, partition-
            # aligned with attn_q: acc_n rows h*64:(h+1)*64 = head h
            # numerator (attn^T), acc_d same layout for denominators.
            acc_n = {
                blk: persist.tile([P, 512], f32, tag=f"an{blk[0]}{blk[1]}",
                                  name=f"an{blk[0]}{blk[1]}")
                for blk in CHUNKED
            }
            acc_d = {
                blk: persist.tile([P, 512], f32, tag=f"ad{blk[0]}{blk[1]}",
                                  name=f"ad{blk[0]}{blk[1]}")
                for blk in CHUNKED
            }

            with (
                tc.tile_pool(name="xt", bufs=2) as xpool,
                tc.tile_pool(name="rope", bufs=2) as rpool,
                tc.tile_pool(name="expp", bufs=4) as epool,
                tc.tile_pool(name="ob", bufs=4) as opool,
                tc.tile_pool(name="norm", bufs=1) as npool,
                tc.tile_pool(name="sc_ps", bufs=2, space="PSUM") as scps,
                tc.tile_pool(name="ps", bufs=4, space="PSUM") as psring,
            ):
                # ---- input DMAs ------------------------------------
                # each dma_start rides ~one HW ring (~30 GB/s), so the
                # first-wave transfers are split into per-slab pieces
                # spread across all four engine queues for parallel rings.
                nc.sync.dma_start(wqk_sb[:], w_qk[:])
                xts = []
                xt0 = xpool.tile([P, 8, 512], bf16, tag="xt", name="xt")
                nc.gpsimd.dma_start(xt0[:], xT[0])
                xts.append(xt0)
                nc.gpsimd.memset(vaug[:, :, :, 0:64], 1.0)
                for hp in range(2):
                    for t in range(4):
                        nc.gpsimd.memset(kpad[hp][0][t][64:128, :], 0.0)
                        nc.gpsimd.memset(kpad[hp][1][t][0:64, :], 0.0)

                # ---- unit emitters ----------------------------------
                def fm_chain(xt, dst, cc, drain):
                    """q or stacked-k feature-major chain -> dst (SBUF).
                    cc: column chunk in wqk ([q_hp0 | k_hp0 | q_hp1 | k_hp1])."""
                    ps = psring.tile([P, 512], f32, tag="b", name="wk")
                    for dc in range(8):
                        nc.tensor.matmul(
                            ps[:],
                            lhsT=wqk_sb[:, dc, cc * P : (cc + 1) * P],
                            rhs=xt[:, dc, :],
                            start=(dc == 0),
                            stop=(dc == 7),
                        )
                    drain(dst[:], ps[:])

                def v_pair(xt, tq, half):
                    """two T-128 blocks of v for all 4 heads -> vaug."""
                    psv = psring.tile([P, 512], f32, tag="b", name="wkv")
                    for t4 in (2 * half, 2 * half + 1):
                        off = (t4 % 2) * 256
                        for dc in range(8):
                            nc.tensor.matmul(
                                psv[:, off : off + 256],
                                lhsT=xt[:, dc, t4 * P : (t4 + 1) * P],
                                rhs=wv_sb[:, dc, :],
                                start=(dc == 0),
                                stop=(dc == 7),
                            )
                    tki = tq * 4 + 2 * half
                    nc.scalar.copy(
                        vaug[:, tki : tki + 2, :, 64:128],
                        psv.rearrange("p (t h e) -> p t h e", t=2, e=64),
                    )

                first_rope = [True]

                def rope_mats(base, tq):
                    hs = slice(tq * 512, (tq + 1) * 512)
                    rot = rpool.tile([P, 512], bf16, tag="rot", name="rot")
                    for blk in range(4):
                        s = (blk ^ 1) * 32
                        eng = nc.sync if blk % 2 == 0 else nc.gpsimd
                        eng.dma_start(
                            rot[blk * 32 : (blk + 1) * 32, :],
                            base[s : s + 32, :],
                        )
                        if first_rope[0]:
                            # second-wave input kicks: queued behind the
                            # dep-gated rot DMA above, so they enter the
                            # DMA rings only once wqk/x0 are ~done
                            first_rope[0] = False
                            hs0 = slice(0, 512)
                            nc.sync.dma_start(cos_sb[:, hs0], cos2[:, hs0])
                            nc.gpsimd.dma_start(sin_sb[:, hs0], sin2[:, hs0])
                            nc.sync.dma_start(wv_sb[:], w_v[:])
                    t1 = rpool.tile([P, 512], bf16, tag="t1", name="t1")
                    nc.vector.tensor_mul(t1[:], base[:], cos_sb[:, hs])
                    nc.vector.tensor_mul(rot[:], rot[:], sin_sb[:, hs])
                    return t1, rot

                def rope(base, tq):
                    """RoPE in place on a [128,512] stacked bf16 tile."""
                    t1, rot = rope_mats(base, tq)
                    nc.vector.tensor_add(base[:], t1[:], rot[:])

                def rope_k(ktmp, hp, tq):
                    """RoPE stacked k into the per-head zero-padded tiles."""
                    t1, rot = rope_mats(ktmp, tq)
                    nc.vector.tensor_add(
                        kpad[hp][0][tq][0:64, :], t1[0:64, :], rot[0:64, :]
                    )
                    nc.vector.tensor_add(
                        kpad[hp][1][tq][64:128, :], t1[64:128, :], rot[64:128, :]
                    )

                # attention step machinery: sc is emitted one tk ahead of
                # the pv flush so the ACT exp stream stays dense.
                pending = []  # list of (hp, tq, tk, ex, pv_pair, start, stop)

                def flush_pending():
                    while pending:
                        emit_pv(*pending.pop(0))

                def emit_pv(hp, tq, tk, ex, pvp, start, stop):
                    for h in range(2):
                        nc.tensor.matmul(
                            pvp[h][:],
                            lhsT=vaug[:, tk, hp * 2 + h, :],
                            rhs=ex[:, h * 512 : (h + 1) * 512],
                            start=start,
                            stop=stop,
                        )
                    if stop:
                        accum_chunk(hp, tq, tk, pvp)

                cur_pv = {}  # blk -> [pv0, pv1] while a chunk is in flight

                def att_step(hp, tq, tk, start, stop):
                    blk = (hp, tq)
                    sc = scps.tile([P, 1024], f32, tag="sc", name="sc")
                    ko = (tk % 4) * P
                    for h in range(2):
                        nc.tensor.matmul(
                            sc[:, h * 512 : (h + 1) * 512],
                            lhsT=kpad[hp][h][tk // 4][:, ko : ko + P],
                            rhs=q_q[hp][tq][:],
                            start=True,
                            stop=True,
                        )
                    ex = epool.tile([P, 1024], bf16, tag="e", name="e")
                    nc.scalar.activation(ex[:], sc[:], Exp, scale=0.125)
                    if _DBG and (hp, tq, tk) == (1, 2, 0):
                        for i, tsrc in ((0, q_q[1][2]), (1, kpad[1][0][0])):
                            db = opool.tile([P, 512], f32, tag="ob", name="db")
                            nc.vector.tensor_copy(db[:], tsrc[:])
                            nc.sync.dma_start(dbg[i], db[:])
                        for i in range(2):
                            db = opool.tile([P, 512], f32, tag="ob", name="db")
                            nc.vector.tensor_copy(
                                db[:], sc[:, i * 512 : (i + 1) * 512]
                            )
                            nc.sync.dma_start(dbg[2 + i], db[:])
                        for i in range(2):
                            db = opool.tile([P, 512], f32, tag="ob", name="db")
                            nc.vector.tensor_copy(
                                db[:], ex[:, i * 512 : (i + 1) * 512]
                            )
                            nc.sync.dma_start(dbg[4 + i], db[:])
                    if start:
                        cur_pv[blk] = [
                            psring.tile([P, 512], f32, tag="b", name=f"pv{h}")
                            for h in range(2)
                        ]
                    if pending:
                        emit_pv(*pending.pop(0))
                    pending.append((hp, tq, tk, ex, cur_pv[blk], start, stop))
                    if not _SKEW:
                        flush_pending()

                acc_init = set()

                def accum_chunk(hp, tq, tk, pvp):
                    """fold a finished pv chunk into the SBUF accum; called
                    from emit_pv when the stop pv lands, so the skew pipeline
                    never breaks at chunk boundaries."""
                    blk = (hp, tq)
                    an, ad = acc_n[blk], acc_d[blk]
                    for h in range(2):
                        hb = slice(h * 64, (h + 1) * 64)
                        if blk not in acc_init:
                            nc.vector.tensor_copy(an[hb, :], pvp[h][64:128, :])
                            nc.vector.tensor_copy(ad[hb, :], pvp[h][0:64, :])
                        else:
                            nc.vector.tensor_add(
                                an[hb, :], an[hb, :], pvp[h][64:128, :]
                            )
                            nc.vector.tensor_add(
                                ad[hb, :], ad[hb, :], pvp[h][0:64, :]
                            )
                    acc_init.add(blk)
                    if tk == 15:
                        norm_chunked(hp, tq)

                step_fifo = []  # (hp, tq, tk, start, stop)

                def push_chunk(hp, tq, tk0, ln):
                    for i in range(ln):
                        step_fifo.append(
                            (hp, tq, tk0 + i, i == 0, i == ln - 1)
                        )

                def pop_steps(n):
                    for _ in range(n):
                        if not step_fifo:
                            return
                        hp, tq, tk, st, sp = step_fifo.pop(0)
                        att_step(hp, tq, tk, start=st, stop=sp)

                def norm_post(hp, tq):
                    if _DBG:
                        nc.gpsimd.dma_start(
                            dbg[8 + 4 * hp + tq], attn_q[hp][tq][:].bitcast(f32)
                        )

                normed = set()

                def norm_chunked(hp, tq):
                    blk = (hp, tq)
                    rc2 = npool.tile([P, 512], f32, tag="rc2", name="rc2")
                    # single full-tile recip: the custom-DVE op is base-0 only
                    nc.vector.reciprocal_approx_fast(rc2[:], acc_d[blk][:])
                    for h in range(2):
                        hb = slice(h * 64, (h + 1) * 64)
                        nc.vector.tensor_mul(
                            attn_q[hp][tq][hb, :], acc_n[blk][hb, :], rc2[hb, :]
                        )
                    normed.add(blk)
                    norm_post(hp, tq)

                def outproj_unit(b, tqc):
                    row = b * 4 + tqc
                    for d2 in range(2):
                        po = psring.tile([P, 512], f32, tag="b", name="po")
                        for hp in range(2):
                            nc.tensor.matmul(
                                po[:],
                                lhsT=attn_q[hp][b][:, tqc * P : (tqc + 1) * P],
                                rhs=wo_sb[:, hp, d2 * 512 : (d2 + 1) * 512],
                                start=(hp == 0),
                                stop=(hp == 1),
                            )
                        ob = opool.tile([P, 512], bf16, tag="ob", name="ob")
                        if d2 == 0:
                            nc.vector.tensor_copy(ob[:], po[:])
                        else:
                            nc.scalar.copy(ob[:], po[:])
                        seng = nc.sync if d2 == 0 else nc.gpsimd
                        seng.dma_start(
                            out[row * P : (row + 1) * P, d2 * 512 : (d2 + 1) * 512],
                            ob[:],
                        )

                def drain_act(dst, ps):
                    nc.scalar.copy(dst, ps)

                def drain_dve(dst, ps):
                    nc.vector.tensor_copy(dst, ps)

                # ---- schedule ---------------------------------------
                # Per-quarter rounds emit the projection chains with
                # backlogged attention steps interleaved (so the ACT exp
                # stream never starves while the PE runs chains), then
                # push the chunks whose (q, kpad-quarter) inputs now
                # exist.  Chunks are 8-tk where availability allows.
                ROUND_PUSH = [
                    [(0, 0, 0, 4), (1, 0, 0, 4)],
                    [(0, 0, 4, 4), (1, 0, 4, 4), (0, 1, 0, 8), (1, 1, 0, 8)],
                    [(0, 0, 8, 4), (1, 0, 8, 4), (0, 1, 8, 4), (1, 1, 8, 4),
                     (0, 2, 0, 8), (1, 2, 0, 8)],
                    [(0, 0, 12, 4), (1, 0, 12, 4), (0, 1, 12, 4),
                     (1, 1, 12, 4), (0, 3, 0, 8), (1, 3, 0, 8), (0, 2, 8, 8)],
                ]
                for j in range(4):
                    xt = xts[j]
                    ktmps = []
                    for hp in range(2):
                        kt = rpool.tile([P, 512], bf16, tag="kt", name="kt")
                        fm_chain(xt, kt, 2 * hp + 1, drain_act)
                        pop_steps(2)
                        ktmps.append(kt)
                    for hp in range(2):
                        rope_k(ktmps[hp], hp, j)
                    if j + 1 < 4:
                        xtn = xpool.tile([P, 8, 512], bf16, tag="xt", name="xt")
                        nc.sync.dma_start(xtn[:], xT[j + 1])
                        xts.append(xtn)
                    for hp in range(2):
                        fm_chain(xt, q_q[hp][j], 2 * hp, drain_dve)
                        pop_steps(2)
                    for hp in range(2):
                        rope(q_q[hp][j], j)
                    v_pair(xt, j, 0)
                    pop_steps(2)
                    v_pair(xt, j, 1)
                    if j + 1 < 4:
                        hsn = slice((j + 1) * 512, (j + 2) * 512)
                        nc.scalar.dma_start(cos_sb[:, hsn], cos2[:, hsn])
                        nc.scalar.dma_start(sin_sb[:, hsn], sin2[:, hsn])
                    if j == 2:
                        nc.scalar.dma_start(wo_sb[:], w_o[:])
                    for chunk in ROUND_PUSH[j]:
                        push_chunk(*chunk)
                    # drain the fifo down to a backlog that covers the
                    # next round's chain section
                    keep = 8 if j < 3 else 0
                    pop_steps(max(0, len(step_fifo) - keep))

                # ---- phase B: remaining chunks + outproj --------------
                for chunk in [
                    (1, 2, 8, 8), (1, 3, 8, 8), (0, 3, 8, 8),
                ]:
                    push_chunk(*chunk)
                op_queue = [
                    (b, tqc) for b in range(4) for tqc in range(4)
                ]
                while step_fifo:
                    pop_steps(4)
                    if op_queue:
                        b = op_queue[0][0]
                        if (0, b) in normed and (1, b) in normed:
                            outproj_unit(*op_queue.pop(0))
                flush_pending()
                while op_queue:
                    outproj_unit(*op_queue.pop(0))

    nc.compile()
    return nc


def _get_module():
    if "nc" not in _CACHE:
        _CACHE["nc"] = _build_module()
    return _CACHE["nc"]


def make_in_maps(x, w_qkv, w_out):
    cos2, sin2 = _rope_tables_np()
    in_maps = []
    for c in range(NCORES):
        b, g = divmod(c, 4)
        q0 = 256 * g
        # column chunks: [q_hp0 | k_hp0 | q_hp1 | k_hp1]
        wqk_c = np.concatenate(
            [
                w_qkv[:, q0 : q0 + 128],
                w_qkv[:, 1024 + q0 : 1024 + q0 + 128],
                w_qkv[:, q0 + 128 : q0 + 256],
                w_qkv[:, 1024 + q0 + 128 : 1024 + q0 + 256],
            ],
            axis=1,
        )
        xt4 = np.ascontiguousarray(
            x[b].T.reshape(8, 128, 4, 512).transpose(2, 1, 0, 3)
        )
        wv_c = w_qkv[:, 2048 + q0 : 2048 + q0 + 256]
        in_maps.append(
            {
                "xT": xt4.astype(BF16),
                "w_qk": np.ascontiguousarray(
                    wqk_c.reshape(8, 128, 512).transpose(1, 0, 2)
                ).astype(BF16),
                "w_v": np.ascontiguousarray(
                    wv_c.reshape(8, 128, 256).transpose(1, 0, 2)
                ).astype(BF16),
                "w_o": np.ascontiguousarray(
                    w_out[q0 : q0 + 256, :].reshape(2, 128, 1024).transpose(1, 0, 2)
                ),
                "cos2": cos2.astype(BF16),
                "sin2": sin2.astype(BF16),
            }
        )
    return in_maps


def combine_outputs(results, b_out):
    out = np.empty((B, T, D), dtype=np.float32)
    for b in range(B):
        acc = results[4 * b]["out"].astype(np.float32)
        for c in range(4 * b + 1, 4 * b + 4):
            acc += results[c]["out"].astype(np.float32)
        out[b] = acc + b_out[None, :]
    return out


def kernel(x, w_qkv, w_out, b_out, _trace=False, _tag=[0]):
    from concourse import bass_utils

    nc = _get_module()
    in_maps = make_in_maps(
        np.asarray(x, dtype=np.float32),
        np.asarray(w_qkv, dtype=np.float32),
        np.asarray(w_out, dtype=np.float32),
    )
    res = bass_utils.run_bass_kernel_spmd(
        nc, in_maps, core_ids=list(range(NCORES)), trace=_trace
    )
    if _trace:
        _CACHE["last_result"] = res
    return combine_outputs(res.results, np.asarray(b_out, dtype=np.float32))


# revision 46
# speedup vs baseline: 1.0669x; 1.0669x over previous
"""Multi-head self-attention (RoPE, eval-mode) Trainium2 Bass kernel.

Problem: B=2, T=2048, D=1024, H=16, d_head=64, fp32 I/O.

Sharding (8 cores): core c handles batch b=c//4 and the 4 heads
[4g, 4g+4) where g=c%4.  QKV/attention are head-local; the output
projection produces a per-core partial (contraction over this core's
256 head-dims) which the host sums across the 4 cores of each batch
and adds b_out.

v2 design (vs the two-phase baseline):
  - The ACT exp stream (4 heads * T^2 = 16.8M elems ~ 110us streaming)
    is the hard wall.  The kernel is restructured so exp starts ~16us
    in instead of ~81us: per-quarter rounds emit the K/V/Q projection
    chains interleaved with attention tk-chunks of the first four
    (hp, tq) blocks, which accumulate PV partials into SBUF so the two
    PSUM pv banks don't serialize in-flight blocks.
  - DMA priority: w_qkv chunk 0 + x quarter 0 are issued first and the
    rest in need order, so the first matmul fires at ~6us not ~22us.
  - Scores are issued as two concurrent K=64 row-group matmuls
    (head 0 rows 0:64, head 1 rows 64:128) instead of zero-padded
    K=128 - halves score cycles; RoPE applies in place on the stacked
    k tiles (kstack == stationary source).
  - Emission skews sc one tk ahead of pv so the exp stream stays dense
    across chunk/block boundaries; per-head norm frees pv banks early.
  - PSUM: sc 2x[128,1024] (4 banks) + pv0/pv1 (2) + work ring 2 = 8.
  - v is computed row-major and stored per head as [ones | v] 128-wide
    stationary tiles: each PV matmul yields softmax denominators
    (partitions 0:64) and attn^T (64:128) in one pass.
  - softmax skips max-subtraction (scores ~ N(0,1), exp safe in fp32)
    and normalizes with the fast DVE reciprocal.
"""

import ml_dtypes
import numpy as np

BF16 = ml_dtypes.bfloat16

B, T, D = 2, 2048, 1024
H = 16
DH = 64
NCORES = 8
P = 128

_CACHE = {}
_DBG = False  # debug build: DMA intermediates of block (0,2) to "dbg"
_SKEW = True  # emit sc one tk ahead of the pv flush (denser ACT stream)
_OPTAIL = False  # emit all outproj units at the tail (diagnostic)
_ALLCHUNK = False  # process dense blocks as 4-tk chunks too (diagnostic)


def _rope_tables_np():
    theta = 1.0 / (10000.0 ** (np.arange(0, DH, 2, dtype=np.float32) / DH))
    angles = np.outer(np.arange(T, dtype=np.float32), theta)  # (T, 32)
    angles = np.concatenate([angles, angles], axis=-1)  # (T, DH)
    cos = np.cos(angles).astype(np.float32)
    sin = np.sin(angles).astype(np.float32)
    cosT = np.ascontiguousarray(cos.T)  # (64, T)
    sinT = np.ascontiguousarray(sin.T)
    sinT_signed = np.concatenate([-sinT[0:32], sinT[32:64]], axis=0)
    cos2 = np.tile(cosT, (2, 1))  # (128, T)
    sin2 = np.tile(sinT_signed, (2, 1))
    return cos2, sin2


def _build_module():
    import concourse.mybir as mybir
    import concourse.tile as tile
    from concourse import bacc

    f32 = mybir.dt.float32
    f32r = mybir.dt.float32r
    bf16 = mybir.dt.bfloat16

    nc = bacc.Bacc("TRN2", target_bir_lowering=False, debug=False)
    xT = nc.dram_tensor("xT", [4, P, 8, 512], bf16, kind="ExternalInput")
    w_qk = nc.dram_tensor("w_qk", [P, 8, 512], bf16, kind="ExternalInput")
    w_v = nc.dram_tensor("w_v", [P, 8, 256], bf16, kind="ExternalInput")
    w_o = nc.dram_tensor("w_o", [P, 2, 1024], f32r, kind="ExternalInput")
    cos2 = nc.dram_tensor("cos2", [P, T], bf16, kind="ExternalInput")
    sin2 = nc.dram_tensor("sin2", [P, T], bf16, kind="ExternalInput")
    out = nc.dram_tensor("out", [T, D], bf16, kind="ExternalOutput")
    dbg = (
        nc.dram_tensor("dbg", [20, P, 512], f32, kind="ExternalOutput")
        if _DBG
        else None
    )

    Exp = mybir.ActivationFunctionType.Exp

    # Blocks in completion order.  The first CHUNKED ones accumulate PV
    # into SBUF in 4-tk chunks paced by quarter availability; the rest
    # run dense (16 tk straight, PV resident in PSUM).
    CHUNKED = [(hp, tq) for tq in range(4) for hp in range(2)]

    with tile.TileContext(nc) as tc:
        with tc.tile_pool(name="persist", bufs=1) as persist:
            wqk_sb = persist.tile([P, 8, 512], bf16)
            wv_sb = persist.tile([P, 8, 256], bf16)
            wo_sb = persist.tile([P, 2, 1024], f32r)
            cos_sb = persist.tile([P, T], bf16)
            sin_sb = persist.tile([P, T], bf16)
            # roped q / stacked roped k, two heads per 128-partition tile
            q_q = [
                [persist.tile([P, 512], bf16, tag=f"q{hp}_{t}", name=f"q{hp}_{t}")
                 for t in range(4)]
                for hp in range(2)
            ]
            # zero-padded roped k per head (head h real rows h*64:h*64+64,
            # other half zero) - K=128 score matmuls need no row groups
            kpad = [
                [
                    [persist.tile([P, 512], bf16, tag=f"k{hp}{h}_{t}",
                                  name=f"k{hp}{h}_{t}")
                     for t in range(4)]
                    for h in range(2)
                ]
                for hp in range(2)
            ]
            # per (tk-tile, head): [ones | v] stationary 128x128
            vaug = persist.tile([P, 16, 4, P], bf16)
            attn_q = [
                [persist.tile([P, 512], f32r, tag=f"at{hp}_{b}", name=f"at{hp}_{b}")
                 for b in range(4)]
                for hp in range(2)
            ]
            # SBUF PV accumulators for the chunked blocks, partition-
            # aligned with attn_q: acc_n rows h*64:(h+1)*64 = head h
            # numerator (attn^T), acc_d same layout for denominators.
            acc_n = {
                blk: persist.tile([P, 512], f32, tag=f"an{blk[0]}{blk[1]}",
                                  name=f"an{blk[0]}{blk[1]}")
                for blk in CHUNKED
            }
            acc_d = {
                blk: persist.tile([P, 512], f32, tag=f"ad{blk[0]}{blk[1]}",
                                  name=f"ad{blk[0]}{blk[1]}")
                for blk in CHUNKED
            }

            with (
                tc.tile_pool(name="xt", bufs=2) as xpool,
                tc.tile_pool(name="rope", bufs=2) as rpool,
                tc.tile_pool(name="expp", bufs=4) as epool,
                tc.tile_pool(name="ob", bufs=4) as opool,
                tc.tile_pool(name="norm", bufs=1) as npool,
                tc.tile_pool(name="sc_ps", bufs=2, space="PSUM") as scps,
                tc.tile_pool(name="pv_ps", bufs=1, space="PSUM") as pvps,
                tc.tile_pool(name="wk_ps", bufs=2, space="PSUM") as wkps,
            ):
                # ---- input DMAs ------------------------------------
                # each dma_start rides ~one HW ring (~30 GB/s), so the
                # first-wave transfers are split into per-slab pieces
                # spread across all four engine queues for parallel rings.
                nc.sync.dma_start(wqk_sb[:], w_qk[:])
                xts = []
                xt0 = xpool.tile([P, 8, 512], bf16, tag="xt", name="xt")
                nc.gpsimd.dma_start(xt0[:], xT[0])
                xts.append(xt0)
                nc.gpsimd.memset(vaug[:, :, :, 0:64], 1.0)
                for hp in range(2):
                    for t in range(4):
                        nc.gpsimd.memset(kpad[hp][0][t][64:128, :], 0.0)
                        nc.gpsimd.memset(kpad[hp][1][t][0:64, :], 0.0)

                # ---- unit emitters ----------------------------------
                def fm_chain(xt, dst, cc, drain):
                    """q or stacked-k feature-major chain -> dst (SBUF).
                    cc: column chunk in wqk ([q_hp0 | k_hp0 | q_hp1 | k_hp1])."""
                    ps = wkps.tile([P, 512], f32, tag="wk", name="wk")
                    for dc in range(8):
                        nc.tensor.matmul(
                            ps[:],
                            lhsT=wqk_sb[:, dc, cc * P : (cc + 1) * P],
                            rhs=xt[:, dc, :],
                            start=(dc == 0),
                            stop=(dc == 7),
                        )
                    drain(dst[:], ps[:])

                def v_pair(xt, tq, half):
                    """two T-128 blocks of v for all 4 heads -> vaug."""
                    psv = wkps.tile([P, 512], f32, tag="wk", name="wkv")
                    for t4 in (2 * half, 2 * half + 1):
                        off = (t4 % 2) * 256
                        for dc in range(8):
                            nc.tensor.matmul(
                                psv[:, off : off + 256],
                                lhsT=xt[:, dc, t4 * P : (t4 + 1) * P],
                                rhs=wv_sb[:, dc, :],
                                start=(dc == 0),
                                stop=(dc == 7),
                            )
                    tki = tq * 4 + 2 * half
                    nc.scalar.copy(
                        vaug[:, tki : tki + 2, :, 64:128],
                        psv.rearrange("p (t h e) -> p t h e", t=2, e=64),
                    )

                first_rope = [True]

                def rope_mats(base, tq):
                    hs = slice(tq * 512, (tq + 1) * 512)
                    rot = rpool.tile([P, 512], bf16, tag="rot", name="rot")
                    for blk in range(4):
                        s = (blk ^ 1) * 32
                        eng = nc.sync if blk % 2 == 0 else nc.gpsimd
                        eng.dma_start(
                            rot[blk * 32 : (blk + 1) * 32, :],
                            base[s : s + 32, :],
                        )
                        if first_rope[0]:
                            # second-wave input kicks: queued behind the
                            # dep-gated rot DMA above, so they enter the
                            # DMA rings only once wqk/x0 are ~done
                            first_rope[0] = False
                            hs0 = slice(0, 512)
                            nc.sync.dma_start(cos_sb[:, hs0], cos2[:, hs0])
                            nc.gpsimd.dma_start(sin_sb[:, hs0], sin2[:, hs0])
                            nc.sync.dma_start(wv_sb[:], w_v[:])
                    t1 = rpool.tile([P, 512], bf16, tag="t1", name="t1")
                    nc.vector.tensor_mul(t1[:], base[:], cos_sb[:, hs])
                    nc.vector.tensor_mul(rot[:], rot[:], sin_sb[:, hs])
                    return t1, rot

                def rope(base, tq):
                    """RoPE in place on a [128,512] stacked bf16 tile."""
                    t1, rot = rope_mats(base, tq)
                    nc.vector.tensor_add(base[:], t1[:], rot[:])

                def rope_k(ktmp, hp, tq):
                    """RoPE stacked k into the per-head zero-padded tiles."""
                    t1, rot = rope_mats(ktmp, tq)
                    nc.vector.tensor_add(
                        kpad[hp][0][tq][0:64, :], t1[0:64, :], rot[0:64, :]
                    )
                    nc.vector.tensor_add(
                        kpad[hp][1][tq][64:128, :], t1[64:128, :], rot[64:128, :]
                    )

                # attention step machinery: sc is emitted one tk ahead of
                # the pv flush so the ACT exp stream stays dense.
                pending = []  # list of (hp, tq, tk, ex, pv_pair, start, stop)

                def flush_pending():
                    while pending:
                        emit_pv(*pending.pop(0))

                def emit_pv(hp, tq, tk, ex, pvp, start, stop):
                    for h in range(2):
                        nc.tensor.matmul(
                            pvp[h][:],
                            lhsT=vaug[:, tk, hp * 2 + h, :],
                            rhs=ex[:, h * 512 : (h + 1) * 512],
                            start=start,
                            stop=stop,
                        )
                    if stop:
                        accum_chunk(hp, tq, tk, pvp)

                cur_pv = {}  # blk -> [pv0, pv1] while a chunk is in flight

                def att_step(hp, tq, tk, start, stop):
                    blk = (hp, tq)
                    sc = scps.tile([P, 1024], f32, tag="sc", name="sc")
                    ko = (tk % 4) * P
                    for h in range(2):
                        nc.tensor.matmul(
                            sc[:, h * 512 : (h + 1) * 512],
                            lhsT=kpad[hp][h][tk // 4][:, ko : ko + P],
                            rhs=q_q[hp][tq][:],
                            start=True,
                            stop=True,
                        )
                    ex = epool.tile([P, 1024], bf16, tag="e", name="e")
                    nc.scalar.activation(ex[:], sc[:], Exp, scale=0.125)
                    if _DBG and (hp, tq, tk) == (1, 2, 0):
                        for i, tsrc in ((0, q_q[1][2]), (1, kpad[1][0][0])):
                            db = opool.tile([P, 512], f32, tag="ob", name="db")
                            nc.vector.tensor_copy(db[:], tsrc[:])
                            nc.sync.dma_start(dbg[i], db[:])
                        for i in range(2):
                            db = opool.tile([P, 512], f32, tag="ob", name="db")
                            nc.vector.tensor_copy(
                                db[:], sc[:, i * 512 : (i + 1) * 512]
                            )
                            nc.sync.dma_start(dbg[2 + i], db[:])
                        for i in range(2):
                            db = opool.tile([P, 512], f32, tag="ob", name="db")
                            nc.vector.tensor_copy(
                                db[:], ex[:, i * 512 : (i + 1) * 512]
                            )
                            nc.sync.dma_start(dbg[4 + i], db[:])
                    if start:
                        cur_pv[blk] = [
                            pvps.tile([P, 512], f32, tag=f"pv{h}", name=f"pv{h}")
                            for h in range(2)
                        ]
                    if pending:
                        emit_pv(*pending.pop(0))
                    pending.append((hp, tq, tk, ex, cur_pv[blk], start, stop))
                    if not _SKEW:
                        flush_pending()

                acc_init = set()

                def accum_chunk(hp, tq, tk, pvp):
                    """fold a finished pv chunk into the SBUF accum; called
                    from emit_pv when the stop pv lands, so the skew pipeline
                    never breaks at chunk boundaries."""
                    blk = (hp, tq)
                    an, ad = acc_n[blk], acc_d[blk]
                    for h in range(2):
                        hb = slice(h * 64, (h + 1) * 64)
                        if blk not in acc_init:
                            nc.vector.tensor_copy(an[hb, :], pvp[h][64:128, :])
                            nc.vector.tensor_copy(ad[hb, :], pvp[h][0:64, :])
                        else:
                            nc.vector.tensor_add(
                                an[hb, :], an[hb, :], pvp[h][64:128, :]
                            )
                            nc.vector.tensor_add(
                                ad[hb, :], ad[hb, :], pvp[h][0:64, :]
                            )
                    acc_init.add(blk)
                    if tk == 15:
                        norm_chunked(hp, tq)

                step_fifo = []  # (hp, tq, tk, start, stop)

                def push_chunk(hp, tq, tk0, ln):
                    for i in range(ln):
                        step_fifo.append(
                            (hp, tq, tk0 + i, i == 0, i == ln - 1)
                        )

                def pop_steps(n):
                    for _ in range(n):
                        if not step_fifo:
                            return
                        hp, tq, tk, st, sp = step_fifo.pop(0)
                        att_step(hp, tq, tk, start=st, stop=sp)

                def norm_post(hp, tq):
                    if _DBG:
                        nc.gpsimd.dma_start(
                            dbg[8 + 4 * hp + tq], attn_q[hp][tq][:].bitcast(f32)
                        )

                normed = set()

                def norm_chunked(hp, tq):
                    blk = (hp, tq)
                    rc2 = npool.tile([P, 512], f32, tag="rc2", name="rc2")
                    # single full-tile recip: the custom-DVE op is base-0 only
                    nc.vector.reciprocal_approx_fast(rc2[:], acc_d[blk][:])
                    for h in range(2):
                        hb = slice(h * 64, (h + 1) * 64)
                        nc.vector.tensor_mul(
                            attn_q[hp][tq][hb, :], acc_n[blk][hb, :], rc2[hb, :]
                        )
                    normed.add(blk)
                    norm_post(hp, tq)

                def outproj_unit(b, tqc):
                    row = b * 4 + tqc
                    for d2 in range(2):
                        po = wkps.tile([P, 512], f32, tag="wk", name="po")
                        for hp in range(2):
                            nc.tensor.matmul(
                                po[:],
                                lhsT=attn_q[hp][b][:, tqc * P : (tqc + 1) * P],
                                rhs=wo_sb[:, hp, d2 * 512 : (d2 + 1) * 512],
                                start=(hp == 0),
                                stop=(hp == 1),
                            )
                        ob = opool.tile([P, 512], bf16, tag="ob", name="ob")
                        if d2 == 0:
                            nc.vector.tensor_copy(ob[:], po[:])
                        else:
                            nc.scalar.copy(ob[:], po[:])
                        seng = nc.sync if d2 == 0 else nc.gpsimd
                        seng.dma_start(
                            out[row * P : (row + 1) * P, d2 * 512 : (d2 + 1) * 512],
                            ob[:],
                        )

                def drain_act(dst, ps):
                    nc.scalar.copy(dst, ps)

                def drain_dve(dst, ps):
                    nc.vector.tensor_copy(dst, ps)

                # ---- schedule ---------------------------------------
                # Per-quarter rounds emit the projection chains with
                # backlogged attention steps interleaved (so the ACT exp
                # stream never starves while the PE runs chains), then
                # push the chunks whose (q, kpad-quarter) inputs now
                # exist.  Chunks are 8-tk where availability allows.
                ROUND_PUSH = [
                    [(0, 0, 0, 4), (1, 0, 0, 4)],
                    [(0, 0, 4, 4), (1, 0, 4, 4), (0, 1, 0, 8), (1, 1, 0, 8)],
                    [(0, 0, 8, 4), (1, 0, 8, 4), (0, 1, 8, 4), (1, 1, 8, 4),
                     (0, 2, 0, 8), (1, 2, 0, 8)],
                    [(0, 0, 12, 4), (1, 0, 12, 4), (0, 1, 12, 4),
                     (1, 1, 12, 4), (0, 3, 0, 8), (1, 3, 0, 8), (0, 2, 8, 8)],
                ]
                for j in range(4):
                    xt = xts[j]
                    ktmps = []
                    for hp in range(2):
                        kt = rpool.tile([P, 512], bf16, tag="kt", name="kt")
                        fm_chain(xt, kt, 2 * hp + 1, drain_act)
                        pop_steps(2)
                        ktmps.append(kt)
                    for hp in range(2):
                        rope_k(ktmps[hp], hp, j)
                    if j + 1 < 4:
                        xtn = xpool.tile([P, 8, 512], bf16, tag="xt", name="xt")
                        nc.sync.dma_start(xtn[:], xT[j + 1])
                        xts.append(xtn)
                    for hp in range(2):
                        fm_chain(xt, q_q[hp][j], 2 * hp, drain_dve)
                        pop_steps(2)
                    for hp in range(2):
                        rope(q_q[hp][j], j)
                    v_pair(xt, j, 0)
                    pop_steps(2)
                    v_pair(xt, j, 1)
                    if j + 1 < 4:
                        hsn = slice((j + 1) * 512, (j + 2) * 512)
                        nc.scalar.dma_start(cos_sb[:, hsn], cos2[:, hsn])
                        nc.scalar.dma_start(sin_sb[:, hsn], sin2[:, hsn])
                    if j == 2:
                        nc.scalar.dma_start(wo_sb[:], w_o[:])
                    for chunk in ROUND_PUSH[j]:
                        push_chunk(*chunk)
                    # drain the fifo down to a backlog that covers the
                    # next round's chain section
                    keep = 8 if j < 3 else 0
                    pop_steps(max(0, len(step_fifo) - keep))

                # ---- phase B: remaining chunks + outproj --------------
                for chunk in [
                    (1, 2, 8, 8), (1, 3, 8, 8), (0, 3, 8, 8),
                ]:
                    push_chunk(*chunk)
                op_queue = [
                    (b, tqc) for b in range(4) for tqc in range(4)
                ]
                while step_fifo:
                    pop_steps(4)
                    if op_queue:
                        b = op_queue[0][0]
                        if (0, b) in normed and (1, b) in normed:
                            outproj_unit(*op_queue.pop(0))
                flush_pending()
                while op_queue:
                    outproj_unit(*op_queue.pop(0))

    nc.compile()
    return nc


def _get_module():
    if "nc" not in _CACHE:
        _CACHE["nc"] = _build_module()
    return _CACHE["nc"]


def make_in_maps(x, w_qkv, w_out):
    cos2, sin2 = _rope_tables_np()
    in_maps = []
    for c in range(NCORES):
        b, g = divmod(c, 4)
        q0 = 256 * g
        # column chunks: [q_hp0 | k_hp0 | q_hp1 | k_hp1]
        wqk_c = np.concatenate(
            [
                w_qkv[:, q0 : q0 + 128],
                w_qkv[:, 1024 + q0 : 1024 + q0 + 128],
                w_qkv[:, q0 + 128 : q0 + 256],
                w_qkv[:, 1024 + q0 + 128 : 1024 + q0 + 256],
            ],
            axis=1,
        )
        xt4 = np.ascontiguousarray(
            x[b].T.reshape(8, 128, 4, 512).transpose(2, 1, 0, 3)
        )
        wv_c = w_qkv[:, 2048 + q0 : 2048 + q0 + 256]
        in_maps.append(
            {
                "xT": xt4.astype(BF16),
                "w_qk": np.ascontiguousarray(
                    wqk_c.reshape(8, 128, 512).transpose(1, 0, 2)
                ).astype(BF16),
                "w_v": np.ascontiguousarray(
                    wv_c.reshape(8, 128, 256).transpose(1, 0, 2)
                ).astype(BF16),
                "w_o": np.ascontiguousarray(
                    w_out[q0 : q0 + 256, :].reshape(2, 128, 1024).transpose(1, 0, 2)
                ),
                "cos2": cos2.astype(BF16),
                "sin2": sin2.astype(BF16),
            }
        )
    return in_maps


def combine_outputs(results, b_out):
    out = np.empty((B, T, D), dtype=np.float32)
    for b in range(B):
        acc = results[4 * b]["out"].astype(np.float32)
        for c in range(4 * b + 1, 4 * b + 4):
            acc += results[c]["out"].astype(np.float32)
        out[b] = acc + b_out[None, :]
    return out


def kernel(x, w_qkv, w_out, b_out, _trace=False, _tag=[0]):
    from concourse import bass_utils

    nc = _get_module()
    in_maps = make_in_maps(
        np.asarray(x, dtype=np.float32),
        np.asarray(w_qkv, dtype=np.float32),
        np.asarray(w_out, dtype=np.float32),
    )
    res = bass_utils.run_bass_kernel_spmd(
        nc, in_maps, core_ids=list(range(NCORES)), trace=_trace
    )
    if _trace:
        _CACHE["last_result"] = res
    return combine_outputs(res.results, np.asarray(b_out, dtype=np.float32))


# revision 47
# speedup vs baseline: 1.0713x; 1.0041x over previous
"""Multi-head self-attention (RoPE, eval-mode) Trainium2 Bass kernel.

Problem: B=2, T=2048, D=1024, H=16, d_head=64, fp32 I/O.

Sharding (8 cores): core c handles batch b=c//4 and the 4 heads
[4g, 4g+4) where g=c%4.  QKV/attention are head-local; the output
projection produces a per-core partial (contraction over this core's
256 head-dims) which the host sums across the 4 cores of each batch
and adds b_out.

v2 design (vs the two-phase baseline):
  - The ACT exp stream (4 heads * T^2 = 16.8M elems ~ 110us streaming)
    is the hard wall.  The kernel is restructured so exp starts ~16us
    in instead of ~81us: per-quarter rounds emit the K/V/Q projection
    chains interleaved with attention tk-chunks of the first four
    (hp, tq) blocks, which accumulate PV partials into SBUF so the two
    PSUM pv banks don't serialize in-flight blocks.
  - DMA priority: w_qkv chunk 0 + x quarter 0 are issued first and the
    rest in need order, so the first matmul fires at ~6us not ~22us.
  - Scores are issued as two concurrent K=64 row-group matmuls
    (head 0 rows 0:64, head 1 rows 64:128) instead of zero-padded
    K=128 - halves score cycles; RoPE applies in place on the stacked
    k tiles (kstack == stationary source).
  - Emission skews sc one tk ahead of pv so the exp stream stays dense
    across chunk/block boundaries; per-head norm frees pv banks early.
  - PSUM: sc 2x[128,1024] (4 banks) + pv0/pv1 (2) + work ring 2 = 8.
  - v is computed row-major and stored per head as [ones | v] 128-wide
    stationary tiles: each PV matmul yields softmax denominators
    (partitions 0:64) and attn^T (64:128) in one pass.
  - softmax skips max-subtraction (scores ~ N(0,1), exp safe in fp32)
    and normalizes with the fast DVE reciprocal.
"""

import ml_dtypes
import numpy as np

BF16 = ml_dtypes.bfloat16

B, T, D = 2, 2048, 1024
H = 16
DH = 64
NCORES = 8
P = 128

_CACHE = {}
_DBG = False  # debug build: DMA intermediates of block (0,2) to "dbg"
_SKEW = True  # emit sc one tk ahead of the pv flush (denser ACT stream)
_OPTAIL = False  # emit all outproj units at the tail (diagnostic)
_ALLCHUNK = False  # process dense blocks as 4-tk chunks too (diagnostic)


def _rope_tables_np():
    theta = 1.0 / (10000.0 ** (np.arange(0, DH, 2, dtype=np.float32) / DH))
    angles = np.outer(np.arange(T, dtype=np.float32), theta)  # (T, 32)
    angles = np.concatenate([angles, angles], axis=-1)  # (T, DH)
    cos = np.cos(angles).astype(np.float32)
    sin = np.sin(angles).astype(np.float32)
    cosT = np.ascontiguousarray(cos.T)  # (64, T)
    sinT = np.ascontiguousarray(sin.T)
    sinT_signed = np.concatenate([-sinT[0:32], sinT[32:64]], axis=0)
    cos2 = np.tile(cosT, (2, 1))  # (128, T)
    sin2 = np.tile(sinT_signed, (2, 1))
    return cos2, sin2


def _build_module():
    import concourse.mybir as mybir
    import concourse.tile as tile
    from concourse import bacc

    f32 = mybir.dt.float32
    f32r = mybir.dt.float32r
    bf16 = mybir.dt.bfloat16

    nc = bacc.Bacc("TRN2", target_bir_lowering=False, debug=False)
    xT = nc.dram_tensor("xT", [4, P, 8, 512], bf16, kind="ExternalInput")
    w_qk = nc.dram_tensor("w_qk", [P, 8, 512], bf16, kind="ExternalInput")
    w_v = nc.dram_tensor("w_v", [P, 8, 256], bf16, kind="ExternalInput")
    w_o = nc.dram_tensor("w_o", [P, 2, 1024], f32r, kind="ExternalInput")
    cos2 = nc.dram_tensor("cos2", [P, T], bf16, kind="ExternalInput")
    sin2 = nc.dram_tensor("sin2", [P, T], bf16, kind="ExternalInput")
    out = nc.dram_tensor("out", [T, D], bf16, kind="ExternalOutput")
    dbg = (
        nc.dram_tensor("dbg", [20, P, 512], f32, kind="ExternalOutput")
        if _DBG
        else None
    )

    Exp = mybir.ActivationFunctionType.Exp

    # Blocks in completion order.  The first CHUNKED ones accumulate PV
    # into SBUF in 4-tk chunks paced by quarter availability; the rest
    # run dense (16 tk straight, PV resident in PSUM).
    CHUNKED = [(hp, tq) for tq in range(4) for hp in range(2)]

    with tile.TileContext(nc) as tc:
        with tc.tile_pool(name="persist", bufs=1) as persist:
            wqk_sb = persist.tile([P, 8, 512], bf16)
            wv_sb = persist.tile([P, 8, 256], bf16)
            wo_sb = persist.tile([P, 2, 1024], f32r)
            cos_sb = persist.tile([P, T], bf16)
            sin_sb = persist.tile([P, T], bf16)
            # roped q / stacked roped k, two heads per 128-partition tile
            q_q = [
                [persist.tile([P, 512], bf16, tag=f"q{hp}_{t}", name=f"q{hp}_{t}")
                 for t in range(4)]
                for hp in range(2)
            ]
            # zero-padded roped k per head (head h real rows h*64:h*64+64,
            # other half zero) - K=128 score matmuls need no row groups
            kpad = [
                [
                    [persist.tile([P, 512], bf16, tag=f"k{hp}{h}_{t}",
                                  name=f"k{hp}{h}_{t}")
                     for t in range(4)]
                    for h in range(2)
                ]
                for hp in range(2)
            ]
            # per (tk-tile, head): [ones | v] stationary 128x128
            vaug = persist.tile([P, 16, 4, P], bf16)
            attn_q = [
                [persist.tile([P, 512], f32r, tag=f"at{hp}_{b}", name=f"at{hp}_{b}")
                 for b in range(4)]
                for hp in range(2)
            ]
            # SBUF PV accumulators for the chunked blocks, partition-
            # aligned with attn_q: acc_n rows h*64:(h+1)*64 = head h
            # numerator (attn^T), acc_d same layout for denominators.
            acc_n = {
                blk: persist.tile([P, 512], f32, tag=f"an{blk[0]}{blk[1]}",
                                  name=f"an{blk[0]}{blk[1]}")
                for blk in CHUNKED
            }
            acc_d = {
                blk: persist.tile([P, 512], f32, tag=f"ad{blk[0]}{blk[1]}",
                                  name=f"ad{blk[0]}{blk[1]}")
                for blk in CHUNKED
            }

            with (
                tc.tile_pool(name="xt", bufs=2) as xpool,
                tc.tile_pool(name="rope", bufs=2) as rpool,
                tc.tile_pool(name="expp", bufs=4) as epool,
                tc.tile_pool(name="ob", bufs=4) as opool,
                tc.tile_pool(name="norm", bufs=1) as npool,
                tc.tile_pool(name="sc_ps", bufs=2, space="PSUM") as scps,
                tc.tile_pool(name="pv_ps", bufs=1, space="PSUM") as pvps,
                tc.tile_pool(name="wk_ps", bufs=2, space="PSUM") as wkps,
            ):
                # ---- input DMAs ------------------------------------
                # each dma_start rides ~one HW ring (~30 GB/s), so the
                # first-wave transfers are split into per-slab pieces
                # spread across all four engine queues for parallel rings.
                nc.sync.dma_start(wqk_sb[:], w_qk[:])
                xts = []
                xt0 = xpool.tile([P, 8, 512], bf16, tag="xt", name="xt")
                nc.gpsimd.dma_start(xt0[:], xT[0])
                xts.append(xt0)
                warm = rpool.tile([P, 512], bf16, tag="kt", name="warm")
                nc.vector.memset(warm[:], 0.0)
                wps = wkps.tile([P, 512], f32, tag="wk", name="warmps")
                for _ in range(28):
                    nc.tensor.matmul(
                        wps[:], lhsT=warm[:, 0:128], rhs=warm[:],
                        start=True, stop=True,
                    )
                nc.gpsimd.memset(vaug[:, :, :, 0:64], 1.0)
                for hp in range(2):
                    for t in range(4):
                        nc.gpsimd.memset(kpad[hp][0][t][64:128, :], 0.0)
                        nc.gpsimd.memset(kpad[hp][1][t][0:64, :], 0.0)

                # ---- unit emitters ----------------------------------
                def fm_chain(xt, dst, cc, drain):
                    """q or stacked-k feature-major chain -> dst (SBUF).
                    cc: column chunk in wqk ([q_hp0 | k_hp0 | q_hp1 | k_hp1])."""
                    ps = wkps.tile([P, 512], f32, tag="wk", name="wk")
                    for dc in range(8):
                        nc.tensor.matmul(
                            ps[:],
                            lhsT=wqk_sb[:, dc, cc * P : (cc + 1) * P],
                            rhs=xt[:, dc, :],
                            start=(dc == 0),
                            stop=(dc == 7),
                        )
                    drain(dst[:], ps[:])

                def v_pair(xt, tq, half):
                    """two T-128 blocks of v for all 4 heads -> vaug."""
                    psv = wkps.tile([P, 512], f32, tag="wk", name="wkv")
                    for t4 in (2 * half, 2 * half + 1):
                        off = (t4 % 2) * 256
                        for dc in range(8):
                            nc.tensor.matmul(
                                psv[:, off : off + 256],
                                lhsT=xt[:, dc, t4 * P : (t4 + 1) * P],
                                rhs=wv_sb[:, dc, :],
                                start=(dc == 0),
                                stop=(dc == 7),
                            )
                    tki = tq * 4 + 2 * half
                    nc.scalar.copy(
                        vaug[:, tki : tki + 2, :, 64:128],
                        psv.rearrange("p (t h e) -> p t h e", t=2, e=64),
                    )

                first_rope = [True]

                def rope_mats(base, tq):
                    hs = slice(tq * 512, (tq + 1) * 512)
                    rot = rpool.tile([P, 512], bf16, tag="rot", name="rot")
                    for blk in range(4):
                        s = (blk ^ 1) * 32
                        eng = nc.sync if blk % 2 == 0 else nc.gpsimd
                        eng.dma_start(
                            rot[blk * 32 : (blk + 1) * 32, :],
                            base[s : s + 32, :],
                        )
                        if first_rope[0]:
                            # second-wave input kicks: queued behind the
                            # dep-gated rot DMA above, so they enter the
                            # DMA rings only once wqk/x0 are ~done
                            first_rope[0] = False
                            hs0 = slice(0, 512)
                            nc.sync.dma_start(cos_sb[:, hs0], cos2[:, hs0])
                            nc.gpsimd.dma_start(sin_sb[:, hs0], sin2[:, hs0])
                            nc.sync.dma_start(wv_sb[:], w_v[:])
                    t1 = rpool.tile([P, 512], bf16, tag="t1", name="t1")
                    nc.vector.tensor_mul(t1[:], base[:], cos_sb[:, hs])
                    nc.vector.tensor_mul(rot[:], rot[:], sin_sb[:, hs])
                    return t1, rot

                def rope(base, tq):
                    """RoPE in place on a [128,512] stacked bf16 tile."""
                    t1, rot = rope_mats(base, tq)
                    nc.vector.tensor_add(base[:], t1[:], rot[:])

                def rope_k(ktmp, hp, tq):
                    """RoPE stacked k into the per-head zero-padded tiles."""
                    t1, rot = rope_mats(ktmp, tq)
                    nc.vector.tensor_add(
                        kpad[hp][0][tq][0:64, :], t1[0:64, :], rot[0:64, :]
                    )
                    nc.vector.tensor_add(
                        kpad[hp][1][tq][64:128, :], t1[64:128, :], rot[64:128, :]
                    )

                # attention step machinery: sc is emitted one tk ahead of
                # the pv flush so the ACT exp stream stays dense.
                pending = []  # list of (hp, tq, tk, ex, pv_pair, start, stop)

                def flush_pending():
                    while pending:
                        emit_pv(*pending.pop(0))

                def emit_pv(hp, tq, tk, ex, pvp, start, stop):
                    for h in range(2):
                        nc.tensor.matmul(
                            pvp[h][:],
                            lhsT=vaug[:, tk, hp * 2 + h, :],
                            rhs=ex[:, h * 512 : (h + 1) * 512],
                            start=start,
                            stop=stop,
                        )
                        if stop:
                            accum_head(hp, tq, tk, h, pvp[h])
                    if stop:
                        acc_init.add((hp, tq))
                        if tk == 15:
                            norm_chunked(hp, tq)

                cur_pv = {}  # blk -> [pv0, pv1] while a chunk is in flight

                def att_step(hp, tq, tk, start, stop):
                    blk = (hp, tq)
                    sc = scps.tile([P, 1024], f32, tag="sc", name="sc")
                    ko = (tk % 4) * P
                    for h in range(2):
                        nc.tensor.matmul(
                            sc[:, h * 512 : (h + 1) * 512],
                            lhsT=kpad[hp][h][tk // 4][:, ko : ko + P],
                            rhs=q_q[hp][tq][:],
                            start=True,
                            stop=True,
                        )
                    ex = epool.tile([P, 1024], bf16, tag="e", name="e")
                    nc.scalar.activation(ex[:], sc[:], Exp, scale=0.125)
                    if _DBG and (hp, tq, tk) == (1, 2, 0):
                        for i, tsrc in ((0, q_q[1][2]), (1, kpad[1][0][0])):
                            db = opool.tile([P, 512], f32, tag="ob", name="db")
                            nc.vector.tensor_copy(db[:], tsrc[:])
                            nc.sync.dma_start(dbg[i], db[:])
                        for i in range(2):
                            db = opool.tile([P, 512], f32, tag="ob", name="db")
                            nc.vector.tensor_copy(
                                db[:], sc[:, i * 512 : (i + 1) * 512]
                            )
                            nc.sync.dma_start(dbg[2 + i], db[:])
                        for i in range(2):
                            db = opool.tile([P, 512], f32, tag="ob", name="db")
                            nc.vector.tensor_copy(
                                db[:], ex[:, i * 512 : (i + 1) * 512]
                            )
                            nc.sync.dma_start(dbg[4 + i], db[:])
                    if start:
                        cur_pv[blk] = [
                            pvps.tile([P, 512], f32, tag=f"pv{h}", name=f"pv{h}")
                            for h in range(2)
                        ]
                    if pending:
                        emit_pv(*pending.pop(0))
                    pending.append((hp, tq, tk, ex, cur_pv[blk], start, stop))
                    if not _SKEW:
                        flush_pending()

                acc_init = set()

                def accum_head(hp, tq, tk, h, pvh):
                    """fold one head of a finished pv chunk into the SBUF
                    accum; fired per-head from emit_pv right after that
                    head's stop matmul so the reads start ASAP."""
                    blk = (hp, tq)
                    an, ad = acc_n[blk], acc_d[blk]
                    hb = slice(h * 64, (h + 1) * 64)
                    if blk not in acc_init:
                        nc.vector.tensor_copy(an[hb, :], pvh[64:128, :])
                        nc.vector.tensor_copy(ad[hb, :], pvh[0:64, :])
                    else:
                        nc.vector.tensor_add(an[hb, :], an[hb, :], pvh[64:128, :])
                        nc.vector.tensor_add(ad[hb, :], ad[hb, :], pvh[0:64, :])

                step_fifo = []  # (hp, tq, tk, start, stop)

                def push_chunk(hp, tq, tk0, ln):
                    for i in range(ln):
                        step_fifo.append(
                            (hp, tq, tk0 + i, i == 0, i == ln - 1)
                        )

                def pop_steps(n):
                    for _ in range(n):
                        if not step_fifo:
                            return
                        hp, tq, tk, st, sp = step_fifo.pop(0)
                        att_step(hp, tq, tk, start=st, stop=sp)

                def norm_post(hp, tq):
                    if _DBG:
                        nc.gpsimd.dma_start(
                            dbg[8 + 4 * hp + tq], attn_q[hp][tq][:].bitcast(f32)
                        )

                normed = set()

                def norm_chunked(hp, tq):
                    blk = (hp, tq)
                    rc2 = npool.tile([P, 512], f32, tag="rc2", name="rc2")
                    # single full-tile recip: the custom-DVE op is base-0 only
                    nc.vector.reciprocal_approx_fast(rc2[:], acc_d[blk][:])
                    for h in range(2):
                        hb = slice(h * 64, (h + 1) * 64)
                        nc.vector.tensor_mul(
                            attn_q[hp][tq][hb, :], acc_n[blk][hb, :], rc2[hb, :]
                        )
                    normed.add(blk)
                    norm_post(hp, tq)

                def outproj_unit(b, tqc):
                    row = b * 4 + tqc
                    for d2 in range(2):
                        po = wkps.tile([P, 512], f32, tag="wk", name="po")
                        for hp in range(2):
                            nc.tensor.matmul(
                                po[:],
                                lhsT=attn_q[hp][b][:, tqc * P : (tqc + 1) * P],
                                rhs=wo_sb[:, hp, d2 * 512 : (d2 + 1) * 512],
                                start=(hp == 0),
                                stop=(hp == 1),
                            )
                        ob = opool.tile([P, 512], bf16, tag="ob", name="ob")
                        if d2 == 0:
                            nc.vector.tensor_copy(ob[:], po[:])
                        else:
                            nc.scalar.copy(ob[:], po[:])
                        seng = nc.sync if d2 == 0 else nc.gpsimd
                        seng.dma_start(
                            out[row * P : (row + 1) * P, d2 * 512 : (d2 + 1) * 512],
                            ob[:],
                        )

                def drain_act(dst, ps):
                    nc.scalar.copy(dst, ps)

                def drain_dve(dst, ps):
                    nc.vector.tensor_copy(dst, ps)

                # ---- schedule ---------------------------------------
                # Per-quarter rounds emit the projection chains with
                # backlogged attention steps interleaved (so the ACT exp
                # stream never starves while the PE runs chains), then
                # push the chunks whose (q, kpad-quarter) inputs now
                # exist.  Chunks are 8-tk where availability allows.
                ROUND_PUSH = [
                    [(0, 0, 0, 4), (1, 0, 0, 4)],
                    [(0, 0, 4, 4), (1, 0, 4, 4), (0, 1, 0, 8), (1, 1, 0, 8)],
                    [(0, 0, 8, 4), (1, 0, 8, 4), (0, 1, 8, 4), (1, 1, 8, 4),
                     (0, 2, 0, 8), (1, 2, 0, 8)],
                    [(0, 0, 12, 4), (1, 0, 12, 4), (0, 1, 12, 4),
                     (1, 1, 12, 4), (0, 3, 0, 8), (1, 3, 0, 8), (0, 2, 8, 8)],
                ]
                for j in range(4):
                    xt = xts[j]
                    ktmps = []
                    for hp in range(2):
                        kt = rpool.tile([P, 512], bf16, tag="kt", name="kt")
                        fm_chain(xt, kt, 2 * hp + 1, drain_act)
                        pop_steps(2)
                        ktmps.append(kt)
                    for hp in range(2):
                        rope_k(ktmps[hp], hp, j)
                    if j + 1 < 4:
                        xtn = xpool.tile([P, 8, 512], bf16, tag="xt", name="xt")
                        nc.sync.dma_start(xtn[:], xT[j + 1])
                        xts.append(xtn)
                    for hp in range(2):
                        fm_chain(xt, q_q[hp][j], 2 * hp, drain_dve)
                        pop_steps(2)
                    for hp in range(2):
                        rope(q_q[hp][j], j)
                    v_pair(xt, j, 0)
                    pop_steps(2)
                    v_pair(xt, j, 1)
                    if j + 1 < 4:
                        hsn = slice((j + 1) * 512, (j + 2) * 512)
                        nc.scalar.dma_start(cos_sb[:, hsn], cos2[:, hsn])
                        nc.scalar.dma_start(sin_sb[:, hsn], sin2[:, hsn])
                    if j == 2:
                        nc.scalar.dma_start(wo_sb[:], w_o[:])
                    for chunk in ROUND_PUSH[j]:
                        push_chunk(*chunk)
                    # drain the fifo down to a backlog that covers the
                    # next round's chain section
                    keep = 12 if j < 3 else 0
                    pop_steps(max(0, len(step_fifo) - keep))

                # ---- phase B: remaining chunks + outproj --------------
                for chunk in [
                    (1, 2, 8, 8), (1, 3, 8, 8), (0, 3, 8, 8),
                ]:
                    push_chunk(*chunk)
                op_queue = [
                    (b, tqc) for b in range(4) for tqc in range(4)
                ]
                while step_fifo:
                    pop_steps(4)
                    if len(op_queue) > 6:
                        b = op_queue[0][0]
                        if (0, b) in normed and (1, b) in normed:
                            outproj_unit(*op_queue.pop(0))
                flush_pending()
                # held-back units of already-normed blocks keep the PE
                # busy (and HAM warm) through the final accum/norm chain
                while op_queue:
                    outproj_unit(*op_queue.pop(0))

    nc.compile()
    return nc


def _get_module():
    if "nc" not in _CACHE:
        _CACHE["nc"] = _build_module()
    return _CACHE["nc"]


def make_in_maps(x, w_qkv, w_out):
    cos2, sin2 = _rope_tables_np()
    in_maps = []
    for c in range(NCORES):
        b, g = divmod(c, 4)
        q0 = 256 * g
        # column chunks: [q_hp0 | k_hp0 | q_hp1 | k_hp1]
        wqk_c = np.concatenate(
            [
                w_qkv[:, q0 : q0 + 128],
                w_qkv[:, 1024 + q0 : 1024 + q0 + 128],
                w_qkv[:, q0 + 128 : q0 + 256],
                w_qkv[:, 1024 + q0 + 128 : 1024 + q0 + 256],
            ],
            axis=1,
        )
        xt4 = np.ascontiguousarray(
            x[b].T.reshape(8, 128, 4, 512).transpose(2, 1, 0, 3)
        )
        wv_c = w_qkv[:, 2048 + q0 : 2048 + q0 + 256]
        in_maps.append(
            {
                "xT": xt4.astype(BF16),
                "w_qk": np.ascontiguousarray(
                    wqk_c.reshape(8, 128, 512).transpose(1, 0, 2)
                ).astype(BF16),
                "w_v": np.ascontiguousarray(
                    wv_c.reshape(8, 128, 256).transpose(1, 0, 2)
                ).astype(BF16),
                "w_o": np.ascontiguousarray(
                    w_out[q0 : q0 + 256, :].reshape(2, 128, 1024).transpose(1, 0, 2)
                ),
                "cos2": cos2.astype(BF16),
                "sin2": sin2.astype(BF16),
            }
        )
    return in_maps


def combine_outputs(results, b_out):
    out = np.empty((B, T, D), dtype=np.float32)
    for b in range(B):
        acc = results[4 * b]["out"].astype(np.float32)
        for c in range(4 * b + 1, 4 * b + 4):
            acc += results[c]["out"].astype(np.float32)
        out[b] = acc + b_out[None, :]
    return out


def kernel(x, w_qkv, w_out, b_out, _trace=False, _tag=[0]):
    from concourse import bass_utils

    nc = _get_module()
    in_maps = make_in_maps(
        np.asarray(x, dtype=np.float32),
        np.asarray(w_qkv, dtype=np.float32),
        np.asarray(w_out, dtype=np.float32),
    )
    res = bass_utils.run_bass_kernel_spmd(
        nc, in_maps, core_ids=list(range(NCORES)), trace=_trace
    )
    if _trace:
        _CACHE["last_result"] = res
    return combine_outputs(res.results, np.asarray(b_out, dtype=np.float32))
